# revision 13
# baseline (speedup 1.0000x reference)
"""Trainium2 Bass kernel for nn_DCGN_5239860101881.

Math (verified against the reference numerically): the DCGN adjacency is
diagonal with diag == 1.0 in fp32, so each propagate collapses to
  out[b] = S * (sum_batch(node_conv(x)) @ W) + bias     (S = 360 / 120)
and the full output is 64 identical [40, 10] blocks.  The only
computation touching the big x tensor is x.sum(axis=0).

Distribution: shard the node axis (1080 = 8 * 135) across 8 cores; each
core streams its [64, 135, 512] slice (16.9 MiB) from HBM and runs the
tiny replicated tail producing 5 of the 40 distinct output rows.

Streaming layout (per core, host-prepacked node-major, group-major):
  xpe [8, 128, 8*256]  f[0:256)   -> SWDGE cast-DMA to bf16, PE reduce
  xda [8, 128, 8*128]  f[256:384) -> HWDGE(sync) fp32, DVE reduce
  xdb [8, 128, 8*128]  f[384:512) -> SWDGE cast-DMA to bf16, PE reduce
  xsl [128, 64*28]     leftover nodes 128..134, one flat HWDGE DMA
Ring byte ratios (gpsimd 12.6M, sync 4.2M, scalar ~1M) make xda/xsl land
early and xdb last, so layer-1 work for f[0:384) hides under the stream.
The leftover-node chain runs first on DVE and its DRAM-roundtrip uses
the scalar queue, so the sync ring starts streaming xda immediately.

Batch reduction: PE accumulates identity-transposes of 2 (resp 4)
batches per matmul in bf16 (psum fp32); DVE does one tensor_reduce per
group.  Tail matmuls run with bf16 operands; prop1_b enters pm1 via a
rank-1 ones x b1 matmul issued before the stream even starts.
"""

import numpy as np
import ml_dtypes

BF16 = ml_dtypes.bfloat16

B, N, F = 64, 1080, 512
H1, H2, NCLS = 784, 28, 10
P = 3
NCORES = 8
SLICE_N = N // NCORES            # 135
NW = SLICE_N // P                # 45 layer-1 windows per core
S2 = NW // P                     # 15 layer-2 windows per core
CR = S2 // P                     # 5 classifier rows per core
GB = 8                           # batches per stream group
NG = B // GB                     # 8 groups
FPE = 256                        # PE share, phase A: f[0:256)
FDA = 128                        # DVE share:         f[256:384)
FPB = 128                        # PE share, phase B: f[384:512)
H1A = 448                        # pm1a columns (4 x 112)
H1B = H1 - H1A                   # pm1b columns (3 x 112)

_CACHE = {}


def _build_bass():
    import concourse.mybir as mybir
    from concourse import bacc
    from concourse.tile import TileContext

    f32 = mybir.dt.float32
    bf = mybir.dt.bfloat16
    AX = mybir.AxisListType
    Gelu = mybir.ActivationFunctionType.Gelu
    Ident = mybir.ActivationFunctionType.Identity
    Copy = mybir.ActivationFunctionType.Copy

    nc = bacc.Bacc("TRN2", target_bir_lowering=False, debug=False,
                   num_devices=NCORES)

    xpe = nc.dram_tensor("xpe", [NG, 128, GB * FPE], f32, kind="ExternalInput")
    xda = nc.dram_tensor("xda", [NG, 128, GB * FDA], f32, kind="ExternalInput")
    xdb = nc.dram_tensor("xdb", [NG, 128, GB * FPB], f32, kind="ExternalInput")
    xsl = nc.dram_tensor("xsl", [128, B * 28], f32, kind="ExternalInput")
    eyeb = nc.dram_tensor("eyeb", [128, 128], bf, kind="ExternalInput")
    w1pat = nc.dram_tensor("w1pat", [128, F], f32, kind="ExternalInput")
    w1patl = nc.dram_tensor("w1patl", [7, F], f32, kind="ExternalInput")
    selb = nc.dram_tensor("selb", [128, NW], bf, kind="ExternalInput")
    sel2b = nc.dram_tensor("sel2b", [7, NW], bf, kind="ExternalInput")
    p1wb = nc.dram_tensor("p1wb", [128, 4, H1], bf, kind="ExternalInput")
    ones1 = nc.dram_tensor("ones1", [1, NW], f32, kind="ExternalInput")
    b1a = nc.dram_tensor("b1a", [1, H1], f32, kind="ExternalInput")
    w2patb = nc.dram_tensor("w2patb", [NW, H1], bf, kind="ExternalInput")
    sel45b = nc.dram_tensor("sel45b", [NW, S2], bf, kind="ExternalInput")
    p2wb = nc.dram_tensor("p2wb", [112, 7, H2], bf, kind="ExternalInput")
    b2c = nc.dram_tensor("b2c", [H2, 1], f32, kind="ExternalInput")
    cw1b = nc.dram_tensor("cw1b", [H2, P, 32], bf, kind="ExternalInput")
    cb1c = nc.dram_tensor("cb1c", [32, 1], f32, kind="ExternalInput")
    cw2b = nc.dram_tensor("cw2b", [32, NCLS], bf, kind="ExternalInput")
    cb2c = nc.dram_tensor("cb2c", [NCLS, 1], f32, kind="ExternalInput")

    out = nc.dram_tensor("out", [NCLS, CR], f32, kind="ExternalOutput")

    with TileContext(nc) as tc:
        with (
            tc.tile_pool(name="w", bufs=1) as wpool,
            tc.tile_pool(name="spe", bufs=3) as spe,
            tc.tile_pool(name="sda", bufs=3) as sda,
            tc.tile_pool(name="spb", bufs=3) as spb,
            tc.tile_pool(name="left", bufs=1) as lpool,
            tc.tile_pool(name="acc", bufs=1) as apool,
            tc.tile_pool(name="prt", bufs=2) as ppool,
            tc.tile_pool(name="tail", bufs=1) as tpool,
            tc.tile_pool(name="psPX", bufs=1, space="PSUM") as psPX,
            tc.tile_pool(name="psM1", bufs=1, space="PSUM") as psM1,
            tc.tile_pool(name="psS", bufs=2, space="PSUM") as psS,
            tc.tile_pool(name="psT", bufs=1, space="PSUM") as psT,
            tc.tile_pool(name="dram", bufs=1, space="DRAM") as dpool,
        ):
            # ---- weights / small inputs on the scalar (qAct HWDGE) ring,
            # tail-critical ones first ----
            eye_sb = wpool.tile([128, 128], bf)
            nc.scalar.dma_start(out=eye_sb, in_=eyeb.ap())
            w1pat_sb = wpool.tile([128, F], f32)
            nc.scalar.dma_start(out=w1pat_sb, in_=w1pat.ap())
            sel_sb = wpool.tile([128, NW], bf)
            nc.scalar.dma_start(out=sel_sb, in_=selb.ap())
            sel2_sb = wpool.tile([7, NW], bf)
            nc.scalar.dma_start(out=sel2_sb, in_=sel2b.ap())
            p1w_sb = wpool.tile([128, 4, H1], bf)
            nc.scalar.dma_start(out=p1w_sb, in_=p1wb.ap())
            ones1_sb = wpool.tile([1, NW], f32)
            nc.scalar.dma_start(out=ones1_sb, in_=ones1.ap())
            b1a_sb = wpool.tile([1, H1], f32)
            nc.scalar.dma_start(out=b1a_sb, in_=b1a.ap())
            w2pat_sb = wpool.tile([NW, H1], bf)
            nc.scalar.dma_start(out=w2pat_sb, in_=w2patb.ap())
            sel45_sb = wpool.tile([NW, S2], bf)
            nc.scalar.dma_start(out=sel45_sb, in_=sel45b.ap())
            p2w_sb = wpool.tile([112, 7, H2], bf)
            nc.scalar.dma_start(out=p2w_sb, in_=p2wb.ap())
            b2_sb = wpool.tile([H2, 1], f32)
            nc.scalar.dma_start(out=b2_sb, in_=b2c.ap())
            cw1_sb = wpool.tile([H2, P, 32], bf)
            nc.scalar.dma_start(out=cw1_sb, in_=cw1b.ap())
            cb1_sb = wpool.tile([32, 1], f32)
            nc.scalar.dma_start(out=cb1_sb, in_=cb1c.ap())
            cw2_sb = wpool.tile([32, NCLS], bf)
            nc.scalar.dma_start(out=cw2_sb, in_=cw2b.ap())
            cb2_sb = wpool.tile([NCLS, 1], f32)
            nc.scalar.dma_start(out=cb2_sb, in_=cb2c.ap())
            w1patl_sb = wpool.tile([7, F], f32)
            nc.scalar.dma_start(out=w1patl_sb, in_=w1patl.ap())
            llt = lpool.tile([128, B * 28], f32)
            nc.scalar.dma_start(out=llt, in_=xsl.ap())

            # preload the gelu ACT table during the stream
            gdummy = tpool.tile([H2, 1], f32)
            nc.scalar.activation(out=gdummy, in_=b2_sb, func=Gelu)

            # bias enters pm1 as a rank-1 update long before the tail
            pm1a = psM1.tile([NW, H1A], f32, tag="m1a")
            pm1b = psM1.tile([NW, H1B], f32, tag="m1b")
            nc.tensor.matmul(pm1a, ones1_sb, b1a_sb[:, 0:H1A],
                             start=True, stop=False)
            nc.tensor.matmul(pm1b, ones1_sb, b1a_sb[:, H1A:H1],
                             start=True, stop=False)

            # ---- phase A: gpsimd(SWDGE) cast stream + PE reduction ----
            px2 = psPX.tile([128, 2 * FPE], f32, tag="pxA")
            for g in range(NG):
                t = spe.tile([128, GB, FPE], bf, tag="pe")
                nc.gpsimd.dma_start(
                    out=t, in_=xpe.ap()[g].rearrange("p (b f) -> p b f", b=GB))
                for m in range(GB // 2):
                    nc.tensor.matmul(px2, eye_sb, t[:, 2 * m:2 * m + 2, :],
                                     start=(g == 0 and m == 0),
                                     stop=(g == NG - 1 and m == GB // 2 - 1))

            # ---- DVE share: sync(HWDGE) stream + per-group reduce ----
            gda0 = sda.tile([128, GB, FDA], f32, tag="da")
            nc.sync.dma_start(
                out=gda0, in_=xda.ap()[0].rearrange("p (b f) -> p b f", b=GB))
            accA = apool.tile([128, FDA], f32)
            nc.vector.reduce_sum(out=accA,
                                 in_=gda0.rearrange("p b f -> p f b"),
                                 axis=AX.X)
            for g in range(1, NG):
                t = sda.tile([128, GB, FDA], f32, tag="da")
                nc.sync.dma_start(
                    out=t, in_=xda.ap()[g].rearrange("p (b f) -> p b f", b=GB))
                part = ppool.tile([128, FDA], f32, tag="part")
                nc.vector.reduce_sum(out=part,
                                     in_=t.rearrange("p b f -> p f b"),
                                     axis=AX.X)
                nc.vector.tensor_add(out=accA, in0=accA, in1=part)

            # ---- leftover nodes: one reduce + DRAM-roundtrip reshape ----
            accl = apool.tile([128, 28], f32)
            nc.vector.reduce_sum(out=accl,
                                 in_=llt.rearrange("p (b f) -> p f b", f=28),
                                 axis=AX.X)
            scratch = dpool.tile([7 * F], f32)
            nc.sync.dma_start(out=scratch.rearrange("(p f) -> p f", p=128),
                              in_=accl)
            lt7 = lpool.tile([7, F], f32)
            nc.sync.dma_start(out=lt7,
                              in_=scratch.rearrange("(n f) -> n f", n=7))
            yl = lpool.tile([7, F], bf)
            nc.vector.tensor_mul(out=yl, in0=lt7, in1=w1patl_sb)

            # ---- ymain for f[0:384) + layer-1 partials (hidden under xdb) --
            ymain = apool.tile([128, F], bf)
            t_pe = apool.tile([128, FPE], f32)
            nc.vector.reduce_sum(out=t_pe,
                                 in_=px2.rearrange("p (q f) -> p f q", q=2),
                                 axis=AX.X)
            nc.vector.tensor_mul(out=ymain[:, 0:FPE], in0=t_pe,
                                 in1=w1pat_sb[:, 0:FPE])
            nc.vector.tensor_mul(out=ymain[:, FPE:FPE + FDA], in0=accA,
                                 in1=w1pat_sb[:, FPE:FPE + FDA])

            hsT = apool.tile([128, 4, NW], bf)
            for fc in range(3):
                ph = psS.tile([128, NW], f32, tag="ph")
                nc.tensor.matmul(ph, ymain[:, fc * 128:(fc + 1) * 128], sel_sb,
                                 start=True, stop=False)
                nc.tensor.matmul(ph, yl[:, fc * 128:(fc + 1) * 128], sel2_sb,
                                 start=False, stop=True)
                nc.scalar.activation(out=hsT[:, fc, :], in_=ph, func=Copy)
                nc.tensor.matmul(pm1a, hsT[:, fc, :], p1w_sb[:, fc, 0:H1A],
                                 start=False, stop=False)
                nc.tensor.matmul(pm1b, hsT[:, fc, :], p1w_sb[:, fc, H1A:H1],
                                 start=False, stop=False)

            # ---- phase B: gpsimd cast stream + PE reduction ----
            pxB = psPX.tile([128, 4 * FPB], f32, tag="pxB")
            for g in range(NG):
                t = spb.tile([128, GB, FPB], bf, tag="pb")
                if g < NG - 1:
                    nc.gpsimd.dma_start(
                        out=t,
                        in_=xdb.ap()[g].rearrange("p (b f) -> p b f", b=GB))
                    for m in range(GB // 4):
                        nc.tensor.matmul(
                            pxB, eye_sb, t[:, 4 * m:4 * m + 4, :],
                            start=(g == 0 and m == 0), stop=False)
                else:
                    # finer sub-DMAs so the last batches' matmuls start early
                    src = xdb.ap()[g].rearrange("p (b f) -> p b f", b=GB)
                    for lo, hi in ((0, 4), (4, 6), (6, 7), (7, 8)):
                        nc.gpsimd.dma_start(out=t[:, lo:hi, :],
                                            in_=src[:, lo:hi, :])
                    nc.tensor.matmul(pxB, eye_sb, t[:, 0:4, :],
                                     start=False, stop=False)
                    nc.tensor.matmul(pxB[:, 0:2 * FPB], eye_sb, t[:, 4:6, :],
                                     start=False, stop=False)
                    nc.tensor.matmul(pxB[:, 2 * FPB:3 * FPB], eye_sb,
                                     t[:, 6, :], start=False, stop=False)
                    nc.tensor.matmul(pxB[:, 3 * FPB:4 * FPB], eye_sb,
                                     t[:, 7, :], start=False, stop=True)

            trB = apool.tile([128, FPB], f32)
            nc.vector.reduce_sum(out=trB,
                                 in_=pxB.rearrange("p (q f) -> p f q", q=4),
                                 axis=AX.X)
            nc.vector.tensor_mul(out=ymain[:, FPE + FDA:F], in0=trB,
                                 in1=w1pat_sb[:, FPE + FDA:F])

            fc = 3
            ph3 = psS.tile([128, NW], f32, tag="ph")
            nc.tensor.matmul(ph3, ymain[:, fc * 128:(fc + 1) * 128], sel_sb,
                             start=True, stop=False)
            nc.tensor.matmul(ph3, yl[:, fc * 128:(fc + 1) * 128], sel2_sb,
                             start=False, stop=True)
            nc.scalar.activation(out=hsT[:, fc, :], in_=ph3, func=Copy)
            nc.tensor.matmul(pm1a, hsT[:, fc, :], p1w_sb[:, fc, 0:H1A],
                             start=False, stop=True)
            nc.tensor.matmul(pm1b, hsT[:, fc, :], p1w_sb[:, fc, H1A:H1],
                             start=False, stop=True)

            # ---- gelu / layer 2, pipelined in 7 chunks of 112 ----
            h1 = tpool.tile([NW, H1], bf)
            y2 = tpool.tile([NW, H1], bf)
            hs2T = tpool.tile([112, 7, S2], bf)
            pm2 = psT.tile([H2, S2], f32, tag="pm2")
            for c in range(7):
                lo = c * 112
                src = (pm1a[:, lo:lo + 112] if c < 4
                       else pm1b[:, lo - H1A:lo - H1A + 112])
                nc.scalar.activation(out=h1[:, lo:lo + 112], in_=src,
                                     func=Gelu)
                nc.vector.tensor_mul(out=y2[:, lo:lo + 112],
                                     in0=h1[:, lo:lo + 112],
                                     in1=w2pat_sb[:, lo:lo + 112])
                phx = psS.tile([128, NW], f32, tag="ph")
                ph2 = phx[0:112, 0:S2]
                nc.tensor.matmul(ph2, y2[:, lo:lo + 112], sel45_sb,
                                 start=True, stop=True)
                nc.vector.tensor_copy(out=hs2T[:, c, :], in_=ph2)
                nc.tensor.matmul(pm2, p2w_sb[:, c, :], hs2T[:, c, :],
                                 start=(c == 0), stop=(c == 6))

            out2T = tpool.tile([H2, S2], bf)
            nc.scalar.activation(out=out2T, in_=pm2, func=Gelu,
                                 bias=b2_sb[:, 0:1], scale=120.0)

            # ---- classifier ----
            o2v = out2T.rearrange("h (r q) -> h r q", q=P)
            pct = psT.tile([32, CR], f32, tag="pc")
            pc1 = pct[0:32, 0:CR]
            for q in range(P):
                nc.tensor.matmul(pc1, cw1_sb[:, q, :], o2v[:, :, q],
                                 start=(q == 0), stop=(q == P - 1))
            c1T = tpool.tile([32, CR], bf)
            nc.scalar.activation(out=c1T, in_=pc1, func=Gelu,
                                 bias=cb1_sb[:, 0:1], scale=1.0)
            pct2 = psT.tile([32, CR], f32, tag="pc")
            pc2 = pct2[0:NCLS, 0:CR]
            nc.tensor.matmul(pc2, cw2_sb, c1T, start=True, stop=True)
            outT = tpool.tile([NCLS, CR], f32)
            nc.scalar.activation(out=outT, in_=pc2, func=Ident,
                                 bias=cb2_sb[:, 0:1], scale=1.0)
            nc.scalar.dma_start(out=out.ap(), in_=outT)

    nc.compile()
    return nc


def _prep_in_maps(inputs):
    x = np.asarray(inputs["x"], dtype=np.float32)
    nc1_w = np.asarray(inputs["nc1_w"], dtype=np.float32)
    prop1_W = np.asarray(inputs["prop1_W"], dtype=np.float32)
    prop1_b = np.asarray(inputs["prop1_b"], dtype=np.float32)
    nc2_w = np.asarray(inputs["nc2_w"], dtype=np.float32)
    prop2_W = np.asarray(inputs["prop2_W"], dtype=np.float32)
    prop2_b = np.asarray(inputs["prop2_b"], dtype=np.float32)
    cls_w1 = np.asarray(inputs["cls_w1"], dtype=np.float32)
    cls_b1 = np.asarray(inputs["cls_b1"], dtype=np.float32)
    cls_w2 = np.asarray(inputs["cls_w2"], dtype=np.float32)
    cls_b2 = np.asarray(inputs["cls_b2"], dtype=np.float32)

    common = {
        "eyeb": np.eye(128, dtype=BF16),
        "w1pat": np.ascontiguousarray(nc1_w[np.arange(128) % P, :]),
        "w1patl": np.ascontiguousarray(nc1_w[(128 + np.arange(7)) % P, :]),
        "selb": (np.arange(128)[:, None] // P
                 == np.arange(NW)[None, :]).astype(BF16),
        "sel2b": ((128 + np.arange(7))[:, None] // P
                  == np.arange(NW)[None, :]).astype(BF16),
        "p1wb": np.ascontiguousarray(
            (np.float32(360.0) * prop1_W).reshape(4, 128, H1)
            .swapaxes(0, 1)).astype(BF16),
        "ones1": np.ones((1, NW), dtype=np.float32),
        "b1a": np.ascontiguousarray(prop1_b.reshape(1, H1)),
        "w2patb": np.ascontiguousarray(
            (64.0 * nc2_w)[np.arange(NW) % P, :]).astype(BF16),
        "sel45b": (np.arange(NW)[:, None] // P
                   == np.arange(S2)[None, :]).astype(BF16),
        "p2wb": np.ascontiguousarray(
            prop2_W.reshape(7, 112, H2).swapaxes(0, 1)).astype(BF16),
        "b2c": np.ascontiguousarray(prop2_b.reshape(H2, 1)),
        "cw1b": np.ascontiguousarray(
            cls_w1.reshape(P, H2, 32).swapaxes(0, 1)).astype(BF16),
        "cb1c": np.ascontiguousarray(cls_b1.reshape(32, 1)),
        "cw2b": cls_w2.astype(BF16),
        "cb2c": np.ascontiguousarray(cls_b2.reshape(NCLS, 1)),
    }
    in_maps = []
    for c in range(NCORES):
        xs = x[:, c * SLICE_N:(c + 1) * SLICE_N, :]
        xm = xs[:, 0:128, :]                             # [B, 128, 512]

        def pack(lo, hi):
            a = xm[:, :, lo:hi]                          # [B, 128, w]
            a = a.reshape(NG, GB, 128, hi - lo).transpose(0, 2, 1, 3)
            return np.ascontiguousarray(a).reshape(NG, 128, GB * (hi - lo))

        xsl = np.ascontiguousarray(
            xs[:, 128:135, :].reshape(B, 128, 28).transpose(1, 0, 2)
            .reshape(128, B * 28))
        in_maps.append({
            "xpe": pack(0, FPE),
            "xda": pack(FPE, FPE + FDA),
            "xdb": pack(FPE + FDA, F),
            "xsl": xsl,
            **common,
        })
    return in_maps


def run(inputs, trace=False):
    from concourse import bass_utils
    if "nc" not in _CACHE:
        _CACHE["nc"] = _build_bass()
    nc = _CACHE["nc"]
    in_maps = _prep_in_maps(inputs)
    res = bass_utils.run_bass_kernel_spmd(
        nc, in_maps, core_ids=list(range(NCORES)), trace=trace)
    outs = [np.asarray(res.results[c]["out"]) for c in range(NCORES)]
    block = np.concatenate([o.T for o in outs], axis=0)       # [40, 10]
    full = np.tile(block, (B, 1)).astype(np.float32)          # [2560, 10]
    return full, res


def kernel(**inputs) -> np.ndarray:
    out, _ = run(inputs, trace=False)
    return out


# revision 14
# speedup vs baseline: 1.1008x; 1.1008x over previous
"""Trainium2 Bass kernel for nn_DCGN_5239860101881.

Math (verified against the reference numerically): the DCGN adjacency is
diagonal with diag == 1.0 in fp32, so each propagate collapses to
  out[b] = S * (sum_batch(node_conv(x)) @ W) + bias     (S = 360 / 120)
and the full output is 64 identical [40, 10] blocks.  The only
computation touching the big x tensor is x.sum(axis=0).

Distribution: shard the node axis (1080 = 8 * 135) across 8 cores; each
core streams its [64, 135, 512] slice (16.9 MiB) from HBM and runs the
tiny replicated tail producing 5 of the 40 distinct output rows.

Streaming layout (per core, host-prepacked node-major, group-major):
  xpe [8, 128, 8*256]  f[0:256)   -> SWDGE cast-DMA to bf16, PE reduce
  xda [8, 128, 8*128]  f[256:384) -> HWDGE(sync) fp32, DVE reduce
  xdb [8, 128, 8*128]  f[384:512) -> SWDGE cast-DMA to bf16, PE reduce
  xsl [128, 64*28]     leftover nodes 128..134, one flat HWDGE DMA
Ring byte ratios (gpsimd 12.6M, sync 4.2M, scalar ~1M) make xda/xsl land
early and xdb last, so layer-1 work for f[0:384) hides under the stream.
The leftover-node chain runs first on DVE and its DRAM-roundtrip uses
the scalar queue, so the sync ring starts streaming xda immediately.

Batch reduction: PE accumulates identity-transposes of 2 (resp 4)
batches per matmul in bf16 (psum fp32); DVE does one tensor_reduce per
group.  Tail matmuls run with bf16 operands; prop1_b enters pm1 via a
rank-1 ones x b1 matmul issued before the stream even starts.
"""

import numpy as np
import ml_dtypes

BF16 = ml_dtypes.bfloat16

B, N, F = 64, 1080, 512
H1, H2, NCLS = 784, 28, 10
P = 3
NCORES = 8
SLICE_N = N // NCORES            # 135
NW = SLICE_N // P                # 45 layer-1 windows per core
S2 = NW // P                     # 15 layer-2 windows per core
CR = S2 // P                     # 5 classifier rows per core
GB = 8                           # batches per stream group
NG = B // GB                     # 8 groups
FPE = 256                        # PE share, phase A: f[0:256)
FDA = 128                        # DVE share:         f[256:384)
FPB = 128                        # PE share, phase B: f[384:512)
H1A = 448                        # pm1a columns (4 x 112)
H1B = H1 - H1A                   # pm1b columns (3 x 112)

_CACHE = {}


def _build_bass():
    import concourse.mybir as mybir
    from concourse import bacc
    from concourse.tile import TileContext

    f32 = mybir.dt.float32
    bf = mybir.dt.bfloat16
    AX = mybir.AxisListType
    Gelu = mybir.ActivationFunctionType.Gelu
    Ident = mybir.ActivationFunctionType.Identity
    Copy = mybir.ActivationFunctionType.Copy

    nc = bacc.Bacc("TRN2", target_bir_lowering=False, debug=False,
                   num_devices=NCORES)

    xpe = nc.dram_tensor("xpe", [NG, 128, GB * FPE], f32, kind="ExternalInput")
    xda = nc.dram_tensor("xda", [NG, 128, GB * FDA], f32, kind="ExternalInput")
    xdb = nc.dram_tensor("xdb", [NG, 128, GB * FPB], f32, kind="ExternalInput")
    xsl = nc.dram_tensor("xsl", [128, B * 28], f32, kind="ExternalInput")
    eyeb = nc.dram_tensor("eyeb", [128, 128], bf, kind="ExternalInput")
    w1pat = nc.dram_tensor("w1pat", [128, F], f32, kind="ExternalInput")
    w1patl = nc.dram_tensor("w1patl", [7, F], f32, kind="ExternalInput")
    selb = nc.dram_tensor("selb", [128, NW], bf, kind="ExternalInput")
    sel2b = nc.dram_tensor("sel2b", [7, NW], bf, kind="ExternalInput")
    p1wb = nc.dram_tensor("p1wb", [128, 4, H1], bf, kind="ExternalInput")
    ones1 = nc.dram_tensor("ones1", [1, NW], f32, kind="ExternalInput")
    b1a = nc.dram_tensor("b1a", [1, H1], f32, kind="ExternalInput")
    w2patb = nc.dram_tensor("w2patb", [NW, H1], bf, kind="ExternalInput")
    sel45b = nc.dram_tensor("sel45b", [NW, S2], bf, kind="ExternalInput")
    p2wb = nc.dram_tensor("p2wb", [112, 7, H2], bf, kind="ExternalInput")
    b2c = nc.dram_tensor("b2c", [H2, 1], f32, kind="ExternalInput")
    cw1b = nc.dram_tensor("cw1b", [H2, P, 32], bf, kind="ExternalInput")
    cb1c = nc.dram_tensor("cb1c", [32, 1], f32, kind="ExternalInput")
    cw2b = nc.dram_tensor("cw2b", [32, NCLS], bf, kind="ExternalInput")
    cb2c = nc.dram_tensor("cb2c", [NCLS, 1], f32, kind="ExternalInput")

    out = nc.dram_tensor("out", [NCLS, CR], f32, kind="ExternalOutput")

    with TileContext(nc) as tc:
        with (
            tc.tile_pool(name="w", bufs=1) as wpool,
            tc.tile_pool(name="spe", bufs=3) as spe,
            tc.tile_pool(name="sda", bufs=3) as sda,
            tc.tile_pool(name="spb", bufs=3) as spb,
            tc.tile_pool(name="left", bufs=1) as lpool,
            tc.tile_pool(name="acc", bufs=1) as apool,
            tc.tile_pool(name="prt", bufs=2) as ppool,
            tc.tile_pool(name="tail", bufs=1) as tpool,
            tc.tile_pool(name="psPX", bufs=1, space="PSUM") as psPX,
            tc.tile_pool(name="psM1", bufs=1, space="PSUM") as psM1,
            tc.tile_pool(name="psS", bufs=2, space="PSUM") as psS,
            tc.tile_pool(name="psT", bufs=1, space="PSUM") as psT,
            tc.tile_pool(name="dram", bufs=1, space="DRAM") as dpool,
        ):
            # ---- weights / small inputs on the scalar (qAct HWDGE) ring,
            # tail-critical ones first ----
            eye_sb = wpool.tile([128, 128], bf)
            nc.scalar.dma_start(out=eye_sb, in_=eyeb.ap())
            w1pat_sb = wpool.tile([128, F], f32)
            nc.scalar.dma_start(out=w1pat_sb, in_=w1pat.ap())
            sel_sb = wpool.tile([128, NW], bf)
            nc.scalar.dma_start(out=sel_sb, in_=selb.ap())
            sel2_sb = wpool.tile([7, NW], bf)
            nc.scalar.dma_start(out=sel2_sb, in_=sel2b.ap())
            p1w_sb = wpool.tile([128, 4, H1], bf)
            nc.scalar.dma_start(out=p1w_sb, in_=p1wb.ap())
            ones1_sb = wpool.tile([1, NW], f32)
            nc.scalar.dma_start(out=ones1_sb, in_=ones1.ap())
            b1a_sb = wpool.tile([1, H1], f32)
            nc.scalar.dma_start(out=b1a_sb, in_=b1a.ap())
            w2pat_sb = wpool.tile([NW, H1], bf)
            nc.scalar.dma_start(out=w2pat_sb, in_=w2patb.ap())
            sel45_sb = wpool.tile([NW, S2], bf)
            nc.scalar.dma_start(out=sel45_sb, in_=sel45b.ap())
            p2w_sb = wpool.tile([112, 7, H2], bf)
            nc.scalar.dma_start(out=p2w_sb, in_=p2wb.ap())
            b2_sb = wpool.tile([H2, 1], f32)
            nc.scalar.dma_start(out=b2_sb, in_=b2c.ap())
            cw1_sb = wpool.tile([H2, P, 32], bf)
            nc.scalar.dma_start(out=cw1_sb, in_=cw1b.ap())
            cb1_sb = wpool.tile([32, 1], f32)
            nc.scalar.dma_start(out=cb1_sb, in_=cb1c.ap())
            cw2_sb = wpool.tile([32, NCLS], bf)
            nc.scalar.dma_start(out=cw2_sb, in_=cw2b.ap())
            cb2_sb = wpool.tile([NCLS, 1], f32)
            nc.scalar.dma_start(out=cb2_sb, in_=cb2c.ap())
            w1patl_sb = wpool.tile([7, F], f32)
            nc.scalar.dma_start(out=w1patl_sb, in_=w1patl.ap())
            llt = lpool.tile([128, B * 28], f32)
            nc.scalar.dma_start(out=llt, in_=xsl.ap())

            # preload the gelu ACT table during the stream
            gdummy = tpool.tile([H2, 1], f32)
            nc.scalar.activation(out=gdummy, in_=b2_sb, func=Gelu)

            # bias enters pm1 as a rank-1 update long before the tail
            pm1a = psM1.tile([NW, H1A], f32, tag="m1a")
            pm1b = psM1.tile([NW, H1B], f32, tag="m1b")
            nc.tensor.matmul(pm1a, ones1_sb, b1a_sb[:, 0:H1A],
                             start=True, stop=False)
            nc.tensor.matmul(pm1b, ones1_sb, b1a_sb[:, H1A:H1],
                             start=True, stop=False)

            # ---- phase A: gpsimd(SWDGE) cast stream + PE reduction ----
            px2 = psPX.tile([128, 2 * FPE], f32, tag="pxA")
            for g in range(NG):
                t = spe.tile([128, GB, FPE], bf, tag="pe")
                nc.gpsimd.dma_start(
                    out=t, in_=xpe.ap()[g].rearrange("p (b f) -> p b f", b=GB))
                for m in range(GB // 2):
                    nc.tensor.matmul(px2, eye_sb, t[:, 2 * m:2 * m + 2, :],
                                     start=(g == 0 and m == 0),
                                     stop=(g == NG - 1 and m == GB // 2 - 1))

            # ---- DVE share: sync(HWDGE) stream + per-group reduce ----
            gda0 = sda.tile([128, GB, FDA], f32, tag="da")
            nc.sync.dma_start(
                out=gda0, in_=xda.ap()[0].rearrange("p (b f) -> p b f", b=GB))
            accA = apool.tile([128, FDA], f32)
            nc.vector.reduce_sum(out=accA,
                                 in_=gda0.rearrange("p b f -> p f b"),
                                 axis=AX.X)
            for g in range(1, NG):
                t = sda.tile([128, GB, FDA], f32, tag="da")
                nc.sync.dma_start(
                    out=t, in_=xda.ap()[g].rearrange("p (b f) -> p b f", b=GB))
                part = ppool.tile([128, FDA], f32, tag="part")
                nc.vector.reduce_sum(out=part,
                                     in_=t.rearrange("p b f -> p f b"),
                                     axis=AX.X)
                nc.vector.tensor_add(out=accA, in0=accA, in1=part)

            # ---- leftover nodes: one reduce + DRAM-roundtrip reshape ----
            accl = apool.tile([128, 28], f32)
            nc.vector.reduce_sum(out=accl,
                                 in_=llt.rearrange("p (b f) -> p f b", f=28),
                                 axis=AX.X)
            scratch = dpool.tile([7 * F], f32)
            nc.sync.dma_start(out=scratch.rearrange("(p f) -> p f", p=128),
                              in_=accl)
            lt7 = lpool.tile([7, F], f32)
            nc.sync.dma_start(out=lt7,
                              in_=scratch.rearrange("(n f) -> n f", n=7))
            yl = lpool.tile([7, F], bf)
            nc.vector.tensor_mul(out=yl, in0=lt7, in1=w1patl_sb)

            # ---- ymain for f[0:384) + layer-1 partials (hidden under xdb) --
            ymain = apool.tile([128, F], bf)
            t_pe = apool.tile([128, FPE], f32)
            nc.vector.reduce_sum(out=t_pe,
                                 in_=px2.rearrange("p (q f) -> p f q", q=2),
                                 axis=AX.X)
            nc.vector.tensor_mul(out=ymain[:, 0:FPE], in0=t_pe,
                                 in1=w1pat_sb[:, 0:FPE])
            nc.vector.tensor_mul(out=ymain[:, FPE:FPE + FDA], in0=accA,
                                 in1=w1pat_sb[:, FPE:FPE + FDA])

            hsT = apool.tile([128, 4, NW], bf)
            for fc in range(3):
                ph = psS.tile([128, NW], f32, tag="ph")
                nc.tensor.matmul(ph, ymain[:, fc * 128:(fc + 1) * 128], sel_sb,
                                 start=True, stop=False)
                nc.tensor.matmul(ph, yl[:, fc * 128:(fc + 1) * 128], sel2_sb,
                                 start=False, stop=True)
                nc.scalar.activation(out=hsT[:, fc, :], in_=ph, func=Copy)
                nc.tensor.matmul(pm1a, hsT[:, fc, :], p1w_sb[:, fc, 0:H1A],
                                 start=False, stop=False)
                nc.tensor.matmul(pm1b, hsT[:, fc, :], p1w_sb[:, fc, H1A:H1],
                                 start=False, stop=False)

            # ---- phase B: gpsimd cast stream + PE reduction ----
            pxB = psPX.tile([128, 4 * FPB], f32, tag="pxB")
            for g in range(NG):
                t = spb.tile([128, GB, FPB], bf, tag="pb")
                if g < NG - 1:
                    nc.gpsimd.dma_start(
                        out=t,
                        in_=xdb.ap()[g].rearrange("p (b f) -> p b f", b=GB))
                    for m in range(GB // 4):
                        nc.tensor.matmul(
                            pxB, eye_sb, t[:, 4 * m:4 * m + 4, :],
                            start=(g == 0 and m == 0), stop=False)
                else:
                    # finer sub-DMAs so the last batches' matmuls start early
                    src = xdb.ap()[g].rearrange("p (b f) -> p b f", b=GB)
                    for lo, hi in ((0, 4), (4, 6), (6, 7), (7, 8)):
                        nc.gpsimd.dma_start(out=t[:, lo:hi, :],
                                            in_=src[:, lo:hi, :])
                    nc.tensor.matmul(pxB, eye_sb, t[:, 0:4, :],
                                     start=False, stop=False)
                    nc.tensor.matmul(pxB[:, 0:2 * FPB], eye_sb, t[:, 4:6, :],
                                     start=False, stop=False)
                    nc.tensor.matmul(pxB[:, 2 * FPB:3 * FPB], eye_sb,
                                     t[:, 6, :], start=False, stop=False)
                    nc.tensor.matmul(pxB[:, 3 * FPB:4 * FPB], eye_sb,
                                     t[:, 7, :], start=False, stop=True)

            nc.gpsimd.dma_start(out=scratch[0:28], in_=gdummy)

            trB = apool.tile([128, FPB], f32)
            nc.vector.reduce_sum(out=trB,
                                 in_=pxB.rearrange("p (q f) -> p f q", q=4),
                                 axis=AX.X)
            nc.vector.tensor_mul(out=ymain[:, FPE + FDA:F], in0=trB,
                                 in1=w1pat_sb[:, FPE + FDA:F])

            fc = 3
            ph3 = psS.tile([128, NW], f32, tag="ph")
            nc.tensor.matmul(ph3, ymain[:, fc * 128:(fc + 1) * 128], sel_sb,
                             start=True, stop=False)
            nc.tensor.matmul(ph3, yl[:, fc * 128:(fc + 1) * 128], sel2_sb,
                             start=False, stop=True)
            nc.scalar.activation(out=hsT[:, fc, :], in_=ph3, func=Copy)
            nc.tensor.matmul(pm1a, hsT[:, fc, :], p1w_sb[:, fc, 0:H1A],
                             start=False, stop=True)
            nc.tensor.matmul(pm1b, hsT[:, fc, :], p1w_sb[:, fc, H1A:H1],
                             start=False, stop=True)

            # ---- gelu / layer 2, pipelined in 7 chunks of 112 ----
            h1 = tpool.tile([NW, H1], bf)
            y2 = tpool.tile([NW, H1], bf)
            hs2T = tpool.tile([112, 7, S2], bf)
            pm2 = psT.tile([H2, S2], f32, tag="pm2")
            for c in range(7):
                lo = c * 112
                src = (pm1a[:, lo:lo + 112] if c < 4
                       else pm1b[:, lo - H1A:lo - H1A + 112])
                nc.scalar.activation(out=h1[:, lo:lo + 112], in_=src,
                                     func=Gelu)
                nc.vector.tensor_mul(out=y2[:, lo:lo + 112],
                                     in0=h1[:, lo:lo + 112],
                                     in1=w2pat_sb[:, lo:lo + 112])
                phx = psS.tile([128, NW], f32, tag="ph")
                ph2 = phx[0:112, 0:S2]
                nc.tensor.matmul(ph2, y2[:, lo:lo + 112], sel45_sb,
                                 start=True, stop=True)
                nc.vector.tensor_copy(out=hs2T[:, c, :], in_=ph2)
                if c > 0:
                    nc.tensor.matmul(pm2, p2w_sb[:, c - 1, :],
                                     hs2T[:, c - 1, :],
                                     start=(c == 1), stop=False)
            nc.tensor.matmul(pm2, p2w_sb[:, 6, :], hs2T[:, 6, :],
                             start=False, stop=True)

            out2T = tpool.tile([H2, S2], bf)
            nc.scalar.activation(out=out2T, in_=pm2, func=Gelu,
                                 bias=b2_sb[:, 0:1], scale=120.0)

            nc.scalar.dma_start(out=scratch[28:56], in_=gdummy)

            # ---- classifier ----
            o2v = out2T.rearrange("h (r q) -> h r q", q=P)
            pct = psT.tile([32, CR], f32, tag="pc")
            pc1 = pct[0:32, 0:CR]
            for q in range(P):
                nc.tensor.matmul(pc1, cw1_sb[:, q, :], o2v[:, :, q],
                                 start=(q == 0), stop=(q == P - 1))
            c1T = tpool.tile([32, CR], bf)
            nc.scalar.activation(out=c1T, in_=pc1, func=Gelu,
                                 bias=cb1_sb[:, 0:1], scale=1.0)
            pct2 = psT.tile([32, CR], f32, tag="pc")
            pc2 = pct2[0:NCLS, 0:CR]
            nc.tensor.matmul(pc2, cw2_sb, c1T, start=True, stop=True)
            outT = tpool.tile([NCLS, CR], f32)
            nc.scalar.activation(out=outT, in_=pc2, func=Ident,
                                 bias=cb2_sb[:, 0:1], scale=1.0)
            nc.scalar.dma_start(out=out.ap(), in_=outT)

    nc.compile()
    return nc


def _prep_in_maps(inputs):
    x = np.asarray(inputs["x"], dtype=np.float32)
    nc1_w = np.asarray(inputs["nc1_w"], dtype=np.float32)
    prop1_W = np.asarray(inputs["prop1_W"], dtype=np.float32)
    prop1_b = np.asarray(inputs["prop1_b"], dtype=np.float32)
    nc2_w = np.asarray(inputs["nc2_w"], dtype=np.float32)
    prop2_W = np.asarray(inputs["prop2_W"], dtype=np.float32)
    prop2_b = np.asarray(inputs["prop2_b"], dtype=np.float32)
    cls_w1 = np.asarray(inputs["cls_w1"], dtype=np.float32)
    cls_b1 = np.asarray(inputs["cls_b1"], dtype=np.float32)
    cls_w2 = np.asarray(inputs["cls_w2"], dtype=np.float32)
    cls_b2 = np.asarray(inputs["cls_b2"], dtype=np.float32)

    common = {
        "eyeb": np.eye(128, dtype=BF16),
        "w1pat": np.ascontiguousarray(nc1_w[np.arange(128) % P, :]),
        "w1patl": np.ascontiguousarray(nc1_w[(128 + np.arange(7)) % P, :]),
        "selb": (np.arange(128)[:, None] // P
                 == np.arange(NW)[None, :]).astype(BF16),
        "sel2b": ((128 + np.arange(7))[:, None] // P
                  == np.arange(NW)[None, :]).astype(BF16),
        "p1wb": np.ascontiguousarray(
            (np.float32(360.0) * prop1_W).reshape(4, 128, H1)
            .swapaxes(0, 1)).astype(BF16),
        "ones1": np.ones((1, NW), dtype=np.float32),
        "b1a": np.ascontiguousarray(prop1_b.reshape(1, H1)),
        "w2patb": np.ascontiguousarray(
            (64.0 * nc2_w)[np.arange(NW) % P, :]).astype(BF16),
        "sel45b": (np.arange(NW)[:, None] // P
                   == np.arange(S2)[None, :]).astype(BF16),
        "p2wb": np.ascontiguousarray(
            prop2_W.reshape(7, 112, H2).swapaxes(0, 1)).astype(BF16),
        "b2c": np.ascontiguousarray(prop2_b.reshape(H2, 1)),
        "cw1b": np.ascontiguousarray(
            cls_w1.reshape(P, H2, 32).swapaxes(0, 1)).astype(BF16),
        "cb1c": np.ascontiguousarray(cls_b1.reshape(32, 1)),
        "cw2b": cls_w2.astype(BF16),
        "cb2c": np.ascontiguousarray(cls_b2.reshape(NCLS, 1)),
    }
    in_maps = []
    for c in range(NCORES):
        xs = x[:, c * SLICE_N:(c + 1) * SLICE_N, :]
        xm = xs[:, 0:128, :]                             # [B, 128, 512]

        def pack(lo, hi):
            a = xm[:, :, lo:hi]                          # [B, 128, w]
            a = a.reshape(NG, GB, 128, hi - lo).transpose(0, 2, 1, 3)
            return np.ascontiguousarray(a).reshape(NG, 128, GB * (hi - lo))

        xsl = np.ascontiguousarray(
            xs[:, 128:135, :].reshape(B, 128, 28).transpose(1, 0, 2)
            .reshape(128, B * 28))
        in_maps.append({
            "xpe": pack(0, FPE),
            "xda": pack(FPE, FPE + FDA),
            "xdb": pack(FPE + FDA, F),
            "xsl": xsl,
            **common,
        })
    return in_maps


def run(inputs, trace=False):
    from concourse import bass_utils
    if "nc" not in _CACHE:
        _CACHE["nc"] = _build_bass()
    nc = _CACHE["nc"]
    in_maps = _prep_in_maps(inputs)
    res = bass_utils.run_bass_kernel_spmd(
        nc, in_maps, core_ids=list(range(NCORES)), trace=trace)
    outs = [np.asarray(res.results[c]["out"]) for c in range(NCORES)]
    block = np.concatenate([o.T for o in outs], axis=0)       # [40, 10]
    full = np.tile(block, (B, 1)).astype(np.float32)          # [2560, 10]
    return full, res


def kernel(**inputs) -> np.ndarray:
    out, _ = run(inputs, trace=False)
    return out
